# revision 3
# baseline (speedup 1.0000x reference)
"""Dilated LSTM (B=8, T=256, C=1024, H=2048, dilation=4) on 8 trn2 NeuronCores.

v5 = baseline ncfw-AllGather exchange (proven stable) + three changes:
  - psum/gate column layout (c -> local dim 128*(c>>5) + 32*band + (c&31))
    makes the DVE 32x32 block transpose directly produce the hT tile, so the
    gathered [1024, 64] DRAM buffer lands in SBUF with ONE 3-d DMA
    (8 contiguous 128B runs/partition) instead of 4 strided DMAs.
  - proj matmuls for step s+1 are emitted BEFORE rec matmuls of step s, so
    the PE streams the input projection while the AllGather for step s-1 is
    in flight.
  - w_hh contraction rows in natural order (AllGather concatenates by rank).
"""

import numpy as np

B, T, C, H, D = 8, 256, 1024, 2048, 4
NCORES = 8
SLICE = H // NCORES      # 256 h-dims owned per core
TS = T // D              # 64 supersteps
NSEQ = B * D             # 32 sequences
KT_C = C // 128          # 8  K-tiles for the input projection
KT_H = H // 128          # 16 K-tiles for the recurrence
Q = SLICE // 4           # 64

MM_BF16 = True

_CACHE = {}


def _build_nc():
    import concourse.bass as bass
    import concourse.mybir as mybir
    import concourse.tile as tile
    from concourse import bacc

    f32 = mybir.dt.float32
    fmm = mybir.dt.bfloat16 if MM_BF16 else f32
    AF = mybir.ActivationFunctionType

    nc = bacc.Bacc(
        "TRN2",
        target_bir_lowering=False,
        debug=False,
        enable_asserts=False,
        num_devices=NCORES,
    )

    xT = nc.dram_tensor("xT", [KT_C, 128, TS * NSEQ], fmm, kind="ExternalInput")
    wihT = nc.dram_tensor("wihT", [C, 4 * SLICE], fmm, kind="ExternalInput")
    whhT = nc.dram_tensor("whhT", [H, 4 * SLICE], fmm, kind="ExternalInput")
    bias4 = nc.dram_tensor("bias4", [4, SLICE], f32, kind="ExternalInput")
    ind4 = nc.dram_tensor("ind4", [4, 128], f32, kind="ExternalInput")
    out_d = nc.dram_tensor("out", [TS // 8, 128, 8 * Q], f32,
                           kind="ExternalOutput")

    with tile.TileContext(nc) as tc:
        with (
            tc.tile_pool(name="const", bufs=1) as const,
            tc.tile_pool(name="state", bufs=1) as state,
            tc.tile_pool(name="work", bufs=3) as work,
            tc.tile_pool(name="psum", bufs=4, space="PSUM") as psum,
            tc.tile_pool(name="dram", bufs=2, space="DRAM") as dram,
        ):
            x_sb = const.tile([128, KT_C * TS * NSEQ], fmm)
            wih_sb = const.tile([128, KT_C * 4 * SLICE], fmm)
            whh_sb = const.tile([128, KT_H * 4 * SLICE], fmm)
            bias_sb = const.tile([4, SLICE], f32)
            ind_sb = const.tile([4, 128], f32)
            nc.sync.dma_start(ind_sb[:], ind4[:])
            for t in range(KT_C):
                nc.sync.dma_start(
                    x_sb[:, t * (TS * NSEQ):(t + 1) * (TS * NSEQ)], xT[t]
                )
                nc.sync.dma_start(
                    wih_sb[:, t * (4 * SLICE):(t + 1) * (4 * SLICE)],
                    wihT[t * 128:(t + 1) * 128, :],
                )
            for t in range(KT_H):
                nc.sync.dma_start(
                    whh_sb[:, t * (4 * SLICE):(t + 1) * (4 * SLICE)],
                    whhT[t * 128:(t + 1) * 128, :],
                )
            nc.sync.dma_start(bias_sb[:], bias4[:])

            hT_sb = state.tile([128, KT_H * NSEQ], fmm)  # K-tile t at cols 32t
            c_sb = state.tile([128, Q], f32)
            hacc = state.tile([128, 8 * Q], f32)  # 8-step output staging
            nc.gpsimd.memset(hT_sb[:], 0.0)
            nc.gpsimd.memset(c_sb[:], 0.0)

            ps_tiles = [None] * TS

            def emit_proj(s):
                ps = psum.tile([128, SLICE], f32, name=f"ps{s}", tag="ps")
                ps_tiles[s] = ps
                nc.tensor.matmul(
                    ps[:], ind_sb[:], bias_sb[:],
                    start=True, stop=False, skip_group_check=True,
                )
                for t in range(KT_C):
                    lhs = x_sb[:, t * (TS * NSEQ) + s * NSEQ:
                               t * (TS * NSEQ) + (s + 1) * NSEQ]
                    for j in range(4):
                        nc.tensor.matmul(
                            ps[32 * j:32 * (j + 1), :],
                            lhs,
                            wih_sb[:, t * 4 * SLICE + j * SLICE:
                                   t * 4 * SLICE + (j + 1) * SLICE],
                            start=False,
                            stop=False,
                            tile_position=(0, 32 * j),
                            skip_group_check=True,
                        )

            emit_proj(0)
            for s in range(TS):
                if s + 1 < TS:
                    emit_proj(s + 1)

                ps = ps_tiles[s]
                for t in range(KT_H):
                    lhs = hT_sb[:, t * NSEQ:(t + 1) * NSEQ]
                    for j in range(4):
                        nc.tensor.matmul(
                            ps[32 * j:32 * (j + 1), :],
                            lhs,
                            whh_sb[:, t * 4 * SLICE + j * SLICE:
                                   t * 4 * SLICE + (j + 1) * SLICE],
                            start=False,
                            stop=(t == KT_H - 1),
                            tile_position=(0, 32 * j),
                            skip_group_check=True,
                        )

                Qh = Q
                sig = work.tile([128, 3 * Qh], f32, name=f"sig{s}", tag="sig")
                nc.scalar.activation(sig[:], ps[:, 0:3 * Qh], AF.Sigmoid)
                tg = work.tile([128, Qh], f32, name=f"tg{s}", tag="tg")
                nc.scalar.activation(tg[:], ps[:, 3 * Qh:4 * Qh], AF.Tanh)
                t1 = work.tile([128, Qh], f32, name=f"t1{s}", tag="t1")
                nc.vector.tensor_mul(t1[:], sig[:, 0:Qh], tg[:])
                nc.vector.tensor_mul(c_sb[:], sig[:, Qh:2 * Qh], c_sb[:])
                nc.vector.tensor_add(c_sb[:], c_sb[:], t1[:])
                tct = work.tile([128, Qh], f32, name=f"tct{s}", tag="tct")
                nc.scalar.activation(tct[:], c_sb[:], AF.Tanh)
                hs_base = (s % 8) * Qh
                h_sb = hacc[:, hs_base:hs_base + Qh]
                nc.vector.tensor_mul(h_sb, sig[:, 2 * Qh:3 * Qh], tct[:])

                if s % 8 == 7:
                    nc.sync.dma_start(out_d[s // 8], hacc[:])

                if s == TS - 1:
                    break

                # h -> bf16 -> 32x32 block transpose: bt[q, 32u+m] =
                # hT_local[128u+q, m]
                h_mm = work.tile([128, Qh], fmm, name=f"hb{s}", tag="hb")
                nc.vector.tensor_copy(h_mm[:], h_sb)
                bt = work.tile([128, Qh], fmm, name=f"bt{s}", tag="bt")
                nc.vector.transpose(bt[:], h_mm[:])
                cc_in = dram.tile([128, Qh], fmm, name=f"cci{s}", tag="cci")
                nc.sync.dma_start(cc_in[:], bt[:])
                cc_out = dram.tile(
                    [NCORES * 128, Qh], fmm, name=f"cco{s}", tag="cco",
                    addr_space="Shared",
                )
                nc.gpsimd.collective_compute(
                    "AllGather",
                    mybir.AluOpType.bypass,
                    replica_groups=[list(range(NCORES))],
                    ins=[cc_in[:]],
                    outs=[cc_out[:]],
                )
                # cc_out[128k + q, c] -> hT_sb[q, 64k + c], split in two so
                # rec K-tiles 0-7 unblock while windows 4-7 still land
                half_cols = (NCORES // 2) * Qh
                for hf in range(2):
                    nc.sync.dma_start(
                        hT_sb[:, hf * half_cols:(hf + 1) * half_cols]
                        .rearrange("q (k c) -> q k c", k=NCORES // 2),
                        cc_out[hf * 512:(hf + 1) * 512, :]
                        .rearrange("(k q) c -> q k c", q=128),
                    )

    nc.compile()
    return nc


def _col_perm(k):
    """Per-core gate-column order: index (band x, gate g, c) -> w row."""
    goff = np.array([0, H, 3 * H, 2 * H])  # [i, f, o, g] (ref order i,f,g,o)
    xx, gg, cc = np.meshgrid(
        np.arange(4), np.arange(4), np.arange(Q), indexing="ij"
    )
    ell = 128 * (cc >> 5) + 32 * xx + (cc & 31)
    return (goff[gg] + k * SLICE + ell).reshape(-1)


def _host_inputs(x, w_ih, b_ih, w_hh, b_hh):
    x = np.ascontiguousarray(np.asarray(x, dtype=np.float32))
    w_ih = np.asarray(w_ih, dtype=np.float32)
    b_ih = np.asarray(b_ih, dtype=np.float32)
    w_hh = np.asarray(w_hh, dtype=np.float32)
    b_hh = np.asarray(b_hh, dtype=np.float32)

    if MM_BF16:
        import ml_dtypes
        mm_np = ml_dtypes.bfloat16
    else:
        mm_np = np.float32

    xr = x.reshape(B, TS, D, KT_C, 128)
    xr = np.ascontiguousarray(xr.transpose(3, 4, 1, 0, 2))
    xT = xr.reshape(KT_C, 128, TS * NSEQ).astype(mm_np)

    bias = b_ih + b_hh
    ind4 = np.zeros((4, 128), dtype=np.float32)
    for j in range(4):
        ind4[j, 32 * j:32 * (j + 1)] = 1.0

    in_maps = []
    for k in range(NCORES):
        cols = _col_perm(k)
        wihT = np.ascontiguousarray(w_ih[cols, :].T).astype(mm_np)
        whhT = np.ascontiguousarray(w_hh[cols, :].T).astype(mm_np)
        bias4 = np.ascontiguousarray(bias[cols].reshape(4, SLICE))
        in_maps.append({
            "xT": xT, "wihT": wihT, "whhT": whhT,
            "bias4": bias4, "ind4": ind4,
        })
    return in_maps


def kernel(x, w_ih, b_ih, w_hh, b_hh, dilation):
    from concourse.bass_utils import run_bass_kernel_spmd

    assert int(dilation) == D, f"kernel hardcodes dilation={D}, got {dilation}"
    assert tuple(np.shape(x)) == (B, T, C)

    if "nc" not in _CACHE:
        _CACHE["nc"] = _build_nc()
    nc = _CACHE["nc"]

    in_maps = _host_inputs(x, w_ih, b_ih, w_hh, b_hh)

    import time

    t0 = time.perf_counter()
    res = run_bass_kernel_spmd(nc, in_maps, core_ids=list(range(NCORES)))
    _CACHE["last_wall_s"] = time.perf_counter() - t0
    _CACHE["last_exec_ns"] = res.exec_time_ns

    return _assemble([r["out"] for r in res.results])


def _assemble(outs):
    # out_k[chunk, 32x+(b*4+ch), j*Q+c]; s = 8*chunk + j
    o = np.stack(outs)                              # [8, 8, 128, 512]
    o = o.reshape(NCORES, TS // 8, 4, B, D, 8, 2, 32)  # k,cn,x,b,ch,j,u,pp
    o = o.transpose(3, 1, 5, 4, 0, 6, 2, 7)        # b, cn, j, ch, k, u, x, pp
    return np.ascontiguousarray(o.reshape(B, T, H), dtype=np.float32)
